# revision 1
# baseline (speedup 1.0000x reference)
"""Trainium2 Bass kernel for a 2-layer GCN (gnn_message_passing).

Reference computation (all f32 inputs):
    h      = relu(adj @ (x @ W1) + b1)        adj: [N, N], x: [N, F]
    logits = adj @ (h @ W2) + b2
    out    = log_softmax(logits, axis=1)       out: [N, C]

Distribution: 1-D row partition over 8 NeuronCores. Core i owns rows
R0 = i*N/8 .. R0+N/8. Because adj is symmetric (by construction), the
column slice adj[:, rows_i] in natural row-major layout is exactly the
transposed operand adj_i^T the TensorEngine needs as its moving operand,
so no on-chip transpose of adj is ever required.

Per-core plan (single NEFF launch, three AllGathers):
  - dummy AllGather first: absorbs the ~60us first-collective ncfw setup
    while the big adj DMA streams in the background.
  - S_i = x_i @ W1 (local rows only), AllGather S -> full S in SBUF (bf16).
  - adj[:, rows_i] streamed in f32, cast to bf16, kept RESIDENT in SBUF
    (16MB) so layer 2 re-uses it with zero extra HBM traffic.
  - layer 1: hT[f, m] accumulated in PSUM over all 64 k-chunks,
    relu+bias epilogue -> bf16.
  - z_i = h_i @ W2 -> AllGather z (bf16, tiny).
  - layer 2: logitsT[c, m] from resident adj + gathered z, +b2,
    PE-transpose to [m, c], log_softmax on-chip, single output DMA.

kernel(**inputs) takes FULL inputs and returns the FULL [N, C] output.
"""

import numpy as np

import concourse.bass as bass
import concourse.mybir as mybir
import concourse.tile as tile
from concourse import bacc
from concourse.bass_utils import run_bass_kernel_spmd
from concourse.masks import make_identity

NCORES = 8
N_FULL = 8192
NFEAT = 512
NHID = 128
NCLASS = 40
F32 = mybir.dt.float32
BF16 = mybir.dt.bfloat16


def build(n_total: int = N_FULL):
    """Build the SPMD Bass graph for one core (same program on all 8)."""
    M = n_total // NCORES          # rows owned by this core
    K = n_total // 128             # 128-row contraction chunks
    MC = M // 128                  # 128-row output chunks on this core
    MW = min(512, M)               # moving free-dim width for the big matmuls
    MH = M // MW                   # number of row groups of width MW
    KK = 2 if K % 2 == 0 else 1    # adj k-chunks per DMA superchunk
    DF = NFEAT // 128              # feature chunks (4)

    nc = bacc.Bacc(
        "TRN2", target_bir_lowering=False, debug=False,
        enable_asserts=True, num_devices=NCORES,
    )

    xi = nc.dram_tensor("xi", [M, NFEAT], F32, kind="ExternalInput")
    adjc = nc.dram_tensor("adjc", [n_total, M], F32, kind="ExternalInput")
    w1 = nc.dram_tensor("w1", [NFEAT, NHID], F32, kind="ExternalInput")
    b1 = nc.dram_tensor("b1", [NHID, 1], F32, kind="ExternalInput")
    w2 = nc.dram_tensor("w2", [NHID, NCLASS], F32, kind="ExternalInput")
    b2 = nc.dram_tensor("b2", [NCLASS, 1], F32, kind="ExternalInput")
    out_ext = nc.dram_tensor("out", [M, NCLASS], F32, kind="ExternalOutput")

    rg = [list(range(NCORES))]

    with tile.TileContext(nc) as tc:
        with (
            # long-lived resident tiles
            tc.tile_pool(name="resident", bufs=1) as res,
            tc.tile_pool(name="dram", bufs=1, space="DRAM") as dram,
        ):
            adjres = res.tile([128, K * M], BF16)          # adj_i^T, bf16, resident
            sres = res.tile([128, K, NHID], BF16)          # gathered S, k-chunk layout
            zres = res.tile([128, K, NCLASS], BF16)        # gathered z, k-chunk layout
            hT = res.tile([128, M], BF16)                  # layer-1 out, [f, m]
            w1bf = res.tile([128, DF, NHID], BF16)
            w2bf = res.tile([128, NCLASS], BF16)
            b1sb = res.tile([128, 1], F32)
            b2sb = res.tile([NCLASS, 1], F32)
            ident = res.tile([128, 128], F32)
            lTsb = res.tile([NCLASS, M], F32)              # logits^T (+b2)
            osb = res.tile([128, MC, NCLASS], F32)         # final log-softmax out
            zloc = res.tile([128, MC, NCLASS], BF16)

            # collective bounce buffers (internal DRAM)
            dum_in = dram.tile([128, 16], F32)
            dum_out = dram.tile([NCORES * 128, 16], F32, addr_space="Shared")
            s_in = dram.tile([M, NHID], BF16)
            s_out = dram.tile([n_total, NHID], BF16, addr_space="Shared")
            z_in = dram.tile([M, NCLASS], BF16)
            z_out = dram.tile([n_total, NCLASS], BF16, addr_space="Shared")

            # ---- warm-up collective: issued first, runs under the adj DMA ----
            with tc.tile_pool(name="warm", bufs=1) as warm:
                dzero = warm.tile([128, 16], F32)
                nc.gpsimd.memset(dzero[:, :], 0.0)
                nc.gpsimd.dma_start(out=dum_in[:, :], in_=dzero[:, :])
            nc.gpsimd.collective_compute(
                "AllGather", mybir.AluOpType.bypass, replica_groups=rg,
                ins=[dum_in[:, :]], outs=[dum_out[:, :]],
            )

            # ---- constants (gpsimd SWDGE queue: keep the sync/scalar HWDGE
            # FIFOs free for the adj stream) ----
            make_identity(nc, ident[:, :])
            with tc.tile_pool(name="consts", bufs=1) as cst:
                w1st = cst.tile([128, DF, NHID], F32)
                nc.gpsimd.dma_start(
                    out=w1st[:, :, :],
                    in_=w1.ap().rearrange("(a p) f -> p a f", p=128),
                )
                nc.vector.tensor_copy(w1bf[:, :, :], w1st[:, :, :])
                w2st = cst.tile([128, NCLASS], F32)
                nc.gpsimd.dma_start(out=w2st[:, :], in_=w2.ap())
                nc.vector.tensor_copy(w2bf[:, :], w2st[:, :])
                nc.gpsimd.dma_start(out=b1sb[:, :], in_=b1.ap())
                nc.gpsimd.dma_start(out=b2sb[:, :], in_=b2.ap())

            # ---- adj stream: pool allocated BELOW the S-phase pools and
            # DMAs issued first, so the two HWDGE FIFOs (sync + scalar)
            # stream adj from t=0 with no dependence on the S phase ----
            astage_ctx = tc.tile_pool(name="astage", bufs=3)
            astage = astage_ctx.__enter__()
            for kk in range(K // KK):
                ast = astage.tile([128, KK, M], F32, tag="ast")
                dma_eng = nc.sync if kk % 2 == 0 else nc.scalar
                dma_eng.dma_start(
                    out=ast[:, :, :],
                    in_=adjc[kk * KK * 128:(kk + 1) * KK * 128, :].rearrange(
                        "(a p) m -> p a m", p=128
                    ),
                )
                nc.vector.tensor_copy(
                    adjres[:, kk * KK * M:(kk + 1) * KK * M],
                    ast.rearrange("p a m -> p (a m)"),
                )

            # ---- S phase: S_i = x_i @ W1, then AllGather ----
            with (
                tc.tile_pool(name="sph", bufs=2) as sph,
                tc.tile_pool(name="spsum", bufs=2, space="PSUM") as spsum,
                tc.tile_pool(name="xt", bufs=1) as xtp,
            ):
                xT = xtp.tile([128, DF, M], BF16)          # x_i^T in d-chunk layout
                sloc = xtp.tile([128, MC, NHID], BF16)
                for nci in range(MC):
                    xst = sph.tile([128, NFEAT], F32, tag="xst", bufs=1)
                    nc.gpsimd.dma_start(
                        out=xst[:, :], in_=xi[nci * 128:(nci + 1) * 128, :]
                    )
                    for d in range(DF):
                        pt = spsum.tile([128, 128], F32, tag="pt")
                        nc.tensor.transpose(
                            pt[:, :], xst[:, d * 128:(d + 1) * 128], ident[:, :]
                        )
                        nc.vector.tensor_copy(
                            xT[:, d, nci * 128:(nci + 1) * 128], pt[:, :]
                        )
                for nci in range(MC):
                    ps = spsum.tile([128, NHID], F32, tag="ps")
                    for d in range(DF):
                        nc.tensor.matmul(
                            ps[:, :],
                            xT[:, d, nci * 128:(nci + 1) * 128],
                            w1bf[:, d, :],
                            start=(d == 0), stop=(d == DF - 1),
                        )
                    nc.vector.tensor_copy(sloc[:, nci, :], ps[:, :])
                nc.gpsimd.dma_start(
                    out=s_in.rearrange("(a p) f -> p a f", p=128),
                    in_=sloc[:, :, :],
                )
            nc.gpsimd.collective_compute(
                "AllGather", mybir.AluOpType.bypass, replica_groups=rg,
                ins=[s_in[:, :]], outs=[s_out[:, :]],
            )
            nc.gpsimd.dma_start(
                out=sres[:, :, :],
                in_=s_out.rearrange("(k p) f -> p k f", p=128),
            )

            if True:
                # ---- layer 1: hT += S_k^T @ adjT_k ----
                with tc.tile_pool(name="hpsum", bufs=1, space="PSUM") as hp:
                    ph = [hp.tile([128, MW], F32, name=f"ph{m}") for m in range(MH)]
                    for k in range(K):
                        for mh in range(MH):
                            nc.tensor.matmul(
                                ph[mh][:, :],
                                sres[:, k, :],
                                adjres[:, k * M + mh * MW:k * M + (mh + 1) * MW],
                                start=(k == 0), stop=(k == K - 1),
                            )
                    for mh in range(MH):
                        nc.scalar.activation(
                            hT[:, mh * MW:(mh + 1) * MW], ph[mh][:, :],
                            mybir.ActivationFunctionType.Relu,
                            bias=b1sb[:, 0:1], scale=1.0,
                        )

                # ---- z_i = h_i @ W2 ----
                with tc.tile_pool(name="zpsum", bufs=2, space="PSUM") as zp:
                    for mc in range(MC):
                        pz = zp.tile([128, NCLASS], F32, tag="pz")
                        nc.tensor.matmul(
                            pz[:, :],
                            hT[:, mc * 128:(mc + 1) * 128],
                            w2bf[:, :],
                            start=True, stop=True,
                        )
                        nc.vector.tensor_copy(zloc[:, mc, :], pz[:, :])
                nc.gpsimd.dma_start(
                    out=z_in.rearrange("(a p) c -> p a c", p=128),
                    in_=zloc[:, :, :],
                )

                # ---- PE warm-keeper: discardable matmuls that span the
                # z-AllGather gap so HAM keeps the PE at full clock and
                # layer 2 starts warm. Reads hT to order after layer 1. ----
                nwarm = min(K, 40)
                with tc.tile_pool(name="wpsum", bufs=1, space="PSUM") as wp:
                    wps = wp.tile([128, MW], F32)
                    for i in range(nwarm):
                        nc.tensor.matmul(
                            wps[:, :],
                            sres[:, i, :],
                            hT[:, 0:MW] if MW <= M else hT[:, :],
                            start=(i == 0), stop=(i == nwarm - 1),
                        )
                    wscr = astage.tile([128, MW], BF16, name="wscr", bufs=1)
                    nc.vector.tensor_copy(wscr[:, :], wps[:, :])
            astage_ctx.__exit__(None, None, None)
            nc.gpsimd.collective_compute(
                "AllGather", mybir.AluOpType.bypass, replica_groups=rg,
                ins=[z_in[:, :]], outs=[z_out[:, :]],
            )
            nc.gpsimd.dma_start(
                out=zres[:, :, :],
                in_=z_out.rearrange("(k p) c -> p k c", p=128),
            )

            # ---- layer 2: logitsT += z_k^T @ adjT_k ----
            with tc.tile_pool(name="lpsum", bufs=1, space="PSUM") as lp:
                pl = [lp.tile([NCLASS, MW], F32, name=f"pl{m}") for m in range(MH)]
                for k in range(K):
                    for mh in range(MH):
                        nc.tensor.matmul(
                            pl[mh][:, :],
                            zres[:, k, :],
                            adjres[:, k * M + mh * MW:k * M + (mh + 1) * MW],
                            start=(k == 0), stop=(k == K - 1),
                        )
                for mh in range(MH):
                    nc.scalar.activation(
                        lTsb[:, mh * MW:(mh + 1) * MW], pl[mh][:, :],
                        mybir.ActivationFunctionType.Identity,
                        bias=b2sb[:, 0:1], scale=1.0,
                    )

            # ---- log_softmax over classes, batched by activation function so
            # the scalar engine loads each ACT table once (not per block) ----
            with (
                tc.tile_pool(name="smp", bufs=1, space="PSUM") as smp,
                tc.tile_pool(name="sms", bufs=1) as sms,
            ):
                ptrs = [smp.tile([128, NCLASS], F32, name=f"ptr{m}") for m in range(MC)]
                mx = sms.tile([128, MC], F32)
                ssum = sms.tile([128, MC], F32)
                lse = sms.tile([128, MC], F32)
                bias2 = sms.tile([128, MC], F32)
                esc = sms.tile([128, NCLASS], F32)
                for mc in range(MC):
                    nc.tensor.transpose(
                        ptrs[mc][:, :], lTsb[:, mc * 128:(mc + 1) * 128],
                        ident[0:NCLASS, 0:NCLASS],
                    )
                for mc in range(MC):
                    nc.vector.tensor_reduce(
                        mx[:, mc:mc + 1], ptrs[mc][:, :], axis=mybir.AxisListType.X,
                        op=mybir.AluOpType.max, negate=True,
                    )
                for mc in range(MC):
                    nc.scalar.activation(
                        esc[:, :], ptrs[mc][:, :], mybir.ActivationFunctionType.Exp,
                        bias=mx[:, mc:mc + 1], scale=1.0,
                        accum_out=ssum[:, mc:mc + 1],
                    )
                nc.scalar.activation(
                    lse[:, :], ssum[:, :], mybir.ActivationFunctionType.Ln,
                )
                nc.vector.tensor_sub(bias2[:, :], mx[:, :], lse[:, :])
                for mc in range(MC):
                    nc.scalar.activation(
                        osb[:, mc, :], ptrs[mc][:, :],
                        mybir.ActivationFunctionType.Identity,
                        bias=bias2[:, mc:mc + 1], scale=1.0,
                    )
            nc.sync.dma_start(
                out=out_ext.ap().rearrange("(a p) c -> p a c", p=128),
                in_=osb[:, :, :],
            )

    nc.compile()
    return nc


_NC_CACHE = {}


def _get_nc(n_total: int):
    if n_total not in _NC_CACHE:
        _NC_CACHE[n_total] = build(n_total)
    return _NC_CACHE[n_total]


def make_in_maps(x, adj, W1, b1, W2, b2):
    n_total = x.shape[0]
    m = n_total // NCORES
    in_maps = []
    for i in range(NCORES):
        r0 = i * m
        in_maps.append({
            "xi": np.ascontiguousarray(x[r0:r0 + m]),
            "adjc": np.ascontiguousarray(adj[:, r0:r0 + m]),
            "w1": np.ascontiguousarray(W1),
            "b1": np.ascontiguousarray(b1.reshape(NHID, 1)),
            "w2": np.ascontiguousarray(W2),
            "b2": np.ascontiguousarray(b2.reshape(NCLASS, 1)),
        })
    return in_maps


def kernel(x, adj, W1, b1, W2, b2):
    x = np.asarray(x, dtype=np.float32)
    adj = np.asarray(adj, dtype=np.float32)
    W1 = np.asarray(W1, dtype=np.float32)
    b1 = np.asarray(b1, dtype=np.float32)
    W2 = np.asarray(W2, dtype=np.float32)
    b2 = np.asarray(b2, dtype=np.float32)
    nc = _get_nc(x.shape[0])
    in_maps = make_in_maps(x, adj, W1, b1, W2, b2)
    res = run_bass_kernel_spmd(nc, in_maps, list(range(NCORES)))
    return np.concatenate([res.results[i]["out"] for i in range(NCORES)], axis=0)

